# revision 3
# baseline (speedup 1.0000x reference)
"""Min-Euclidean-distance retrieval kernel for Trainium2 (8 NeuronCores).

Reference computation:
    x: [1, 2048, 512], y: [1, 65536, 512] (fp32)
    sq[p, r] = ||x_p||^2 + ||y_r||^2 - 2 <x_p, y_r>
    out = min over (p, r) of sqrt(max(sq, 0))

Sharding: candidate pool (R) split across 8 cores, 8192 candidates each.
Host pre-arranges both GEMM operands partition-major in fp8 so each DMA
moves contiguous per-partition runs and the contraction dim lands on SBUF
partitions with no on-chip transposes.

Per core the hot loop is 64 candidate tiles of [128 cand x 2048 queries].
Engine budget per tile (PE window = 8 DoubleRow matmuls = 1.73 us):
  TensorE:  8 fp8 DoubleRow MMs into a [128, 2048] PSUM tile   1.73 us
  ScalarE:  affine (h = -2*G + y2[r]) on cols 0:1760, bf16     1.73 us
  VectorE:  tensor_scalar affine on cols 1760:2048 (0.51 us)
            + running min acc=min(acc,h) bf16 2x mode (1.23 us) 1.73 us
The y2 bias is applied per candidate-row (PSUM partition) before the min
over tiles; the per-query ||x_p||^2 term commutes with that min and is
added on the host, along with the final min across lanes/cores and the
(monotone) sqrt. fp8 GEMM + bf16 epilogue measure ~1.8e-3 relative error
on the final distance, well inside the 2e-2 tolerance.
"""

import sys

for _p in ("/opt/trn_rl_repo", "/root/.axon_site/_ro/trn_rl_repo"):
    if _p not in sys.path:
        sys.path.append(_p)

import ml_dtypes
import numpy as np

import concourse.bass as bass
import concourse.mybir as mybir
import concourse.tile as tile
from concourse import bacc, bass_utils

P = 2048          # queries
R = 65536         # candidates (full)
D = 512           # feature dim
NCORES = 8
R_LOC = R // NCORES      # 8192 candidates per core
P_CHUNKS = P // 512      # 4 query chunks (one PSUM bank each)
R_TILES = R_LOC // 128   # 64 candidate tiles
K_TILES = D // 128       # 4 contraction tiles (2 DoubleRow passes)

F32 = mybir.dt.float32
BF16 = mybir.dt.bfloat16
MM_DT = mybir.dt.float8e4
MM_NP = ml_dtypes.float8_e4m3
ACC_DT, ACC_NP = BF16, ml_dtypes.bfloat16
# The epilogue runs in bf16 for DVE 2x mode. A constant shift keeps the
# values that matter (near the global min, sq ~ 650 => h ~ 150) in a bf16
# range with quantum ~1; the extra quantization error (~2e-4 on the
# distance) is negligible next to the fp8 GEMM noise.
Y2_SHIFT = np.float32(512.0)
# ScalarE/VectorE split point for the per-tile affine epilogue: ScalarE
# covers [0:FD_A] at 1 elem/lane/cycle @1.2GHz, VectorE tensor_scalar
# covers the rest, keeping both inside the PE's 1.73us/tile window.
FD_A = 1760


def _build_module() -> bass.Bass:
    nc = bacc.Bacc("TRN2", target_bir_lowering=False, debug=False)

    # Host-prepared layouts (partition-major, contiguous per partition):
    #   xt[q, c, k, j]  = x[c*512 + j, k*128 + q]
    #   yt[q, t, k, s]  = y[t*128 + s, k*128 + q]   (t-major: one candidate
    #                     tile = one contiguous 512B-per-partition slice)
    #   y2t[lane, t]    = ||y_r||^2 - Y2_SHIFT for r = t*128 + lane
    xt = nc.dram_tensor("xt", [128, P_CHUNKS, K_TILES, 512], MM_DT,
                        kind="ExternalInput")
    yt = nc.dram_tensor("yt", [128, R_TILES, K_TILES, 128], MM_DT,
                        kind="ExternalInput")
    y2t = nc.dram_tensor("y2t", [128, R_TILES], F32, kind="ExternalInput")
    # acc[lane, p] = min over r-tiles t of (y2[t*128+lane] - 2 G[t*128+lane, p])
    out = nc.dram_tensor("out", [128, P], ACC_DT, kind="ExternalOutput")

    with tile.TileContext(nc) as tc:
        with (
            tc.tile_pool(name="big", bufs=1) as big,
            tc.tile_pool(name="scr", bufs=4) as scr,
            tc.tile_pool(name="psum", bufs=2, space="PSUM") as psum,
        ):
            xt_sb = big.tile([128, P_CHUNKS, K_TILES, 512], MM_DT)
            yt_sb = big.tile([128, R_TILES, K_TILES, 128], MM_DT)
            y2t_sb = big.tile([128, R_TILES], F32)
            acc = big.tile([128, P], ACC_DT)

            # Leading-edge DMAs: only sync (SP) + scalar (Activation) have
            # HWDGE rings (~60 GB/s each), gpsimd drives SWDGE. All of x
            # (1 MB) is needed within the first two tiles, so both HWDGE
            # rings stream x pieces interleaved in MM-consumption order
            # while SWDGE delivers y2t + the first y tiles. Bulk y prefetch
            # follows on whichever ring frees up first.
            for c in (0, 2):
                for kk in (0, 1):
                    nc.scalar.dma_start(xt_sb[:, c, 2 * kk : 2 * kk + 2],
                                        xt.ap()[:, c, 2 * kk : 2 * kk + 2])
            for c in (1, 3):
                for kk in (0, 1):
                    nc.sync.dma_start(xt_sb[:, c, 2 * kk : 2 * kk + 2],
                                      xt.ap()[:, c, 2 * kk : 2 * kk + 2])
            nc.gpsimd.dma_start(y2t_sb[:], y2t.ap())
            for t in range(4):
                nc.gpsimd.dma_start(yt_sb[:, t], yt.ap()[:, t])
            for t4 in range(1, 4):
                nc.sync.dma_start(yt_sb[:, 4 * t4 : 4 * t4 + 4],
                                  yt.ap()[:, 4 * t4 : 4 * t4 + 4])
            for t4 in range(4, 8):
                nc.scalar.dma_start(yt_sb[:, 4 * t4 : 4 * t4 + 4],
                                    yt.ap()[:, 4 * t4 : 4 * t4 + 4])
            for t4 in range(8, 16):
                nc.gpsimd.dma_start(yt_sb[:, 4 * t4 : 4 * t4 + 4],
                                    yt.ap()[:, 4 * t4 : 4 * t4 + 4])

            prev_h = None
            for t in range(R_TILES):
                pt = psum.tile([128, P], F32, name="pt")
                # kk outer => the stationary operand stays loaded across
                # the 4 query chunks; also matches x DMA arrival order.
                for kk in range(K_TILES // 2):
                    for c in range(P_CHUNKS):
                        nc.tensor.matmul(
                            pt[:, c * 512 : (c + 1) * 512],
                            lhsT=yt_sb[:, t, 2 * kk : 2 * kk + 2],
                            rhs=xt_sb[:, c, 2 * kk : 2 * kk + 2],
                            start=(kk == 0),
                            stop=(kk == K_TILES // 2 - 1),
                            perf_mode=mybir.MatmulPerfMode.DoubleRow,
                        )
                bias = y2t_sb[:, t : t + 1]
                dst = acc if t == 0 else scr.tile([128, P], ACC_DT, name="h")
                nc.scalar.activation(
                    out=dst[:, 0:FD_A],
                    in_=pt[:, 0:FD_A],
                    func=mybir.ActivationFunctionType.Identity,
                    bias=bias,
                    scale=-2.0,
                )
                nc.vector.tensor_scalar(
                    out=dst[:, FD_A:P],
                    in0=pt[:, FD_A:P],
                    scalar1=-2.0,
                    scalar2=bias,
                    op0=mybir.AluOpType.mult,
                    op1=mybir.AluOpType.add,
                )
                # Emit the min one tile late so DVE's tensor_scalar (which
                # frees the PSUM buffer) never queues behind it.
                if prev_h is not None:
                    nc.vector.tensor_tensor(
                        out=acc[:], in0=acc[:], in1=prev_h[:],
                        op=mybir.AluOpType.min,
                    )
                prev_h = None if t == 0 else dst
            nc.vector.tensor_tensor(
                out=acc[:], in0=acc[:], in1=prev_h[:], op=mybir.AluOpType.min,
            )
            nc.sync.dma_start(out.ap(), acc[:])
    nc.compile()
    return nc


_module_cache: bass.Bass | None = None


def _get_module() -> bass.Bass:
    global _module_cache
    if _module_cache is None:
        _module_cache = _build_module()
    return _module_cache


def _prepare_inputs(x: np.ndarray, y: np.ndarray):
    """Host-side sharding/layout prep. Returns per-core input maps."""
    # xt[q, c, k, j] = x[c*512 + j, k*128 + q]
    xt4 = x.T.reshape(K_TILES, 128, P_CHUNKS, 512)
    xt = np.ascontiguousarray(xt4.transpose(1, 2, 0, 3).astype(MM_NP))
    in_maps = []
    for cc in range(NCORES):
        yc = y[cc * R_LOC : (cc + 1) * R_LOC]
        # yt[q, t, k, s] = yc[t*128 + s, k*128 + q]
        a = yc.reshape(R_TILES, 128, K_TILES, 128)
        yct = np.ascontiguousarray(a.transpose(3, 0, 2, 1).astype(MM_NP))
        y2 = np.einsum("rd,rd->r", yc, yc, dtype=np.float32) - Y2_SHIFT
        y2t = np.ascontiguousarray(y2.reshape(R_TILES, 128).T)
        in_maps.append({"xt": xt, "yt": yct, "y2t": y2t})
    return in_maps


def _postprocess(x: np.ndarray, accs: np.ndarray) -> np.ndarray:
    """accs: [NCORES, 128, P] partial mins (missing the x2 term)."""
    m = accs.astype(np.float32).min(axis=(0, 1)) + Y2_SHIFT
    x2 = np.einsum("pd,pd->p", x, x, dtype=np.float32)
    sq_min = np.float32((x2 + m).min())
    return np.sqrt(np.maximum(sq_min, np.float32(0.0)), dtype=np.float32)


def kernel(
    predicted_transaction_company: np.ndarray,
    future_transaction_companies_inc_current_data: np.ndarray,
) -> np.ndarray:
    x = np.asarray(predicted_transaction_company, dtype=np.float32)[0]
    y = np.asarray(future_transaction_companies_inc_current_data, dtype=np.float32)[0]

    nc = _get_module()
    in_maps = _prepare_inputs(x, y)
    res = bass_utils.run_bass_kernel_spmd(nc, in_maps, core_ids=list(range(NCORES)))
    accs = np.stack([r["out"] for r in res.results])
    return _postprocess(x, accs)


# revision 5
# speedup vs baseline: 1.1844x; 1.1844x over previous
"""Min-Euclidean-distance retrieval kernel for Trainium2 (8 NeuronCores).

Reference computation:
    x: [1, 2048, 512], y: [1, 65536, 512] (fp32)
    sq[p, r] = ||x_p||^2 + ||y_r||^2 - 2 <x_p, y_r>
    out = min over (p, r) of sqrt(max(sq, 0))

Sharding: candidate pool (R) split across 8 cores, 8192 candidates each.
Host pre-arranges both GEMM operands partition-major in fp8 so each DMA
moves contiguous per-partition runs and the contraction dim lands on SBUF
partitions with no on-chip transposes.

Per core the hot loop is 64 candidate tiles of [128 cand x 2048 queries].
The epilogue (y2 bias + running min over tiles) exceeds what ScalarE
alone can sustain (1.97us/tile vs the PE's 1.73us tile period), so query
columns are split 1536/512 into disjoint PSUM pools / h tiles (Tile's
hazard tracking is tile-granular — shared tiles serialize engines):
  TensorE:  8 fp8 DoubleRow MMs; chunks c0-c2 -> pt_a, c3 -> pt_b
  ScalarE:  h_a = -2*pt_a + y2[r]   (1536 cols; pt_a done at MM#7, so
            this starts one MM early and its 2-period dependency chain
            stays under the PE period)
  VectorE:  h_b = -2*pt_b + y2[r] (tensor_scalar, 512 cols, 0.74us)
            acc_a = min(acc_a, h_a)  (fp16 2x tensor_tensor, 0.96us)
  DMA:      h_b tiles stream raw to DRAM (gpsimd SWDGE queue, 128KB per
            tile ~= 70 GB/s); the host takes the min over those.
The per-query ||x_p||^2 term commutes with the min over candidates and
is added on the host, with the final min across lanes/cores/tiles and
the (monotone) sqrt. fp8 GEMM + fp16 epilogue measure ~1.8e-3 relative
error on the final distance, well inside the 2e-2 tolerance.
"""

import sys

for _p in ("/opt/trn_rl_repo", "/root/.axon_site/_ro/trn_rl_repo"):
    if _p not in sys.path:
        sys.path.append(_p)

import ml_dtypes
import numpy as np

import concourse.bass as bass
import concourse.mybir as mybir
import concourse.tile as tile
from concourse import bacc, bass_utils

P = 2048          # queries
R = 65536         # candidates (full)
D = 512           # feature dim
NCORES = 8
R_LOC = R // NCORES      # 8192 candidates per core
P_CHUNKS = P // 512      # 4 query chunks (one PSUM bank each)
R_TILES = R_LOC // 128   # 64 candidate tiles
K_TILES = D // 128       # 4 contraction tiles (2 DoubleRow passes)
PA = 1536                # query cols on the ScalarE/VectorE-min path
PB = P - PA              # query cols shipped raw to the host

F32 = mybir.dt.float32
MM_DT = mybir.dt.float8e4
MM_NP = ml_dtypes.float8_e4m3
ACC_DT, ACC_NP = mybir.dt.float16, np.float16
# The epilogue runs in fp16. A constant shift keeps the values that matter
# (near the global min, sq ~ 650 => h ~ 150) small; fp16 quantum there is
# ~0.125, negligible next to the fp8 GEMM noise.
Y2_SHIFT = np.float32(512.0)


def _build_module() -> bass.Bass:
    nc = bacc.Bacc("TRN2", target_bir_lowering=False, debug=False)

    # Host-prepared layouts (partition-major, contiguous per partition):
    #   xt[q, c, k, j]  = x[c*512 + j, k*128 + q]
    #   yt[q, t, k, s]  = y[t*128 + s, k*128 + q]   (t-major: one candidate
    #                     tile = one contiguous 512B-per-partition slice)
    #   y2t[lane, t]    = ||y_r||^2 - Y2_SHIFT for r = t*128 + lane
    xt = nc.dram_tensor("xt", [128, P_CHUNKS, K_TILES, 512], MM_DT,
                        kind="ExternalInput")
    yt = nc.dram_tensor("yt", [128, R_TILES, K_TILES, 128], MM_DT,
                        kind="ExternalInput")
    y2t = nc.dram_tensor("y2t", [128, R_TILES], F32, kind="ExternalInput")
    # out[lane, p<PA] = min over r-tiles t of (y2[t*128+lane] - 2 G[.])
    out = nc.dram_tensor("out", [128, PA], ACC_DT, kind="ExternalOutput")
    # hbd[lane, t, j] = y2[t*128+lane] - 2 G[t*128+lane, PA+j]  (no min)
    hbd = nc.dram_tensor("hbd", [128, R_TILES, PB], ACC_DT,
                         kind="ExternalOutput")

    with tile.TileContext(nc) as tc:
        with (
            tc.tile_pool(name="big", bufs=1) as big,
            tc.tile_pool(name="scra", bufs=4) as scra,
            tc.tile_pool(name="scrb", bufs=8) as scrb,
            tc.tile_pool(name="psa", bufs=2, space="PSUM") as psa,
            tc.tile_pool(name="psb", bufs=2, space="PSUM") as psb,
        ):
            xt_sb = big.tile([128, P_CHUNKS, K_TILES, 512], MM_DT)
            yt_sb = big.tile([128, R_TILES, K_TILES, 128], MM_DT)
            y2t_sb = big.tile([128, R_TILES], F32)
            acc_a = big.tile([128, PA], ACC_DT)

            # Leading-edge DMAs: sync (SP) + scalar (Activation) HWDGE rings
            # (~60 GB/s each) carry all of x (needed within two tiles)
            # interleaved in MM-consumption order plus the first y tile;
            # gpsimd SWDGE delivers y2t + y tiles 1-3. Bulk y prefetch is
            # spread over all three queues, staying ahead of the PE.
            nc.sync.dma_start(yt_sb[:, 0], yt.ap()[:, 0])
            for kk in (0, 1):
                nc.scalar.dma_start(xt_sb[:, 0, 2 * kk : 2 * kk + 2],
                                    xt.ap()[:, 0, 2 * kk : 2 * kk + 2])
                nc.sync.dma_start(xt_sb[:, 1, 2 * kk : 2 * kk + 2],
                                  xt.ap()[:, 1, 2 * kk : 2 * kk + 2])
                nc.scalar.dma_start(xt_sb[:, 2, 2 * kk : 2 * kk + 2],
                                    xt.ap()[:, 2, 2 * kk : 2 * kk + 2])
                nc.sync.dma_start(xt_sb[:, 3, 2 * kk : 2 * kk + 2],
                                  xt.ap()[:, 3, 2 * kk : 2 * kk + 2])
            nc.gpsimd.dma_start(y2t_sb[:], y2t.ap())
            for t in range(1, 4):
                nc.gpsimd.dma_start(yt_sb[:, t], yt.ap()[:, t])
            for t4 in range(1, 4):
                nc.sync.dma_start(yt_sb[:, 4 * t4 : 4 * t4 + 4],
                                  yt.ap()[:, 4 * t4 : 4 * t4 + 4])
            for t4 in range(4, 8):
                nc.scalar.dma_start(yt_sb[:, 4 * t4 : 4 * t4 + 4],
                                    yt.ap()[:, 4 * t4 : 4 * t4 + 4])
            for t4 in range(8, 16):
                nc.gpsimd.dma_start(yt_sb[:, 4 * t4 : 4 * t4 + 4],
                                    yt.ap()[:, 4 * t4 : 4 * t4 + 4])

            prev_ha = None
            for t in range(R_TILES):
                pa = psa.tile([128, PA], F32, name="pa")
                pb = psb.tile([128, PB], F32, name="pb")
                # kk outer keeps the stationary operand loaded across
                # chunks; c3 last so pt_a completes at MM#7 and ScalarE
                # starts one MM early.
                for kk in range(K_TILES // 2):
                    for c in range(P_CHUNKS):
                        dst = (pa[:, c * 512 : (c + 1) * 512]
                               if c < 3 else pb[:])
                        nc.tensor.matmul(
                            dst,
                            lhsT=yt_sb[:, t, 2 * kk : 2 * kk + 2],
                            rhs=xt_sb[:, c, 2 * kk : 2 * kk + 2],
                            start=(kk == 0),
                            stop=(kk == K_TILES // 2 - 1),
                            perf_mode=mybir.MatmulPerfMode.DoubleRow,
                        )
                bias = y2t_sb[:, t : t + 1]
                ha = acc_a if t == 0 else scra.tile([128, PA], ACC_DT, name="ha")
                hb = scrb.tile([128, PB], ACC_DT, name="hb")
                nc.scalar.activation(
                    out=ha[:],
                    in_=pa[:],
                    func=mybir.ActivationFunctionType.Identity,
                    bias=bias,
                    scale=-2.0,
                )
                nc.vector.tensor_scalar(
                    out=hb[:],
                    in0=pb[:],
                    scalar1=-2.0,
                    scalar2=bias,
                    op0=mybir.AluOpType.mult,
                    op1=mybir.AluOpType.add,
                )
                nc.gpsimd.dma_start(hbd.ap()[:, t], hb[:])
                # The min runs one tile late so VectorE's tensor_scalar
                # (which frees the psb buffer) never queues behind it.
                if prev_ha is not None:
                    nc.vector.tensor_tensor(
                        out=acc_a[:], in0=acc_a[:], in1=prev_ha[:],
                        op=mybir.AluOpType.min,
                    )
                prev_ha = None if t == 0 else ha
            nc.vector.tensor_tensor(
                out=acc_a[:], in0=acc_a[:], in1=prev_ha[:], op=mybir.AluOpType.min,
            )
            nc.sync.dma_start(out.ap(), acc_a[:])
    nc.compile()
    return nc


_module_cache: bass.Bass | None = None


def _get_module() -> bass.Bass:
    global _module_cache
    if _module_cache is None:
        _module_cache = _build_module()
    return _module_cache


def _prepare_inputs(x: np.ndarray, y: np.ndarray):
    """Host-side sharding/layout prep. Returns per-core input maps."""
    # xt[q, c, k, j] = x[c*512 + j, k*128 + q]
    xt4 = x.T.reshape(K_TILES, 128, P_CHUNKS, 512)
    xt = np.ascontiguousarray(xt4.transpose(1, 2, 0, 3).astype(MM_NP))
    in_maps = []
    for cc in range(NCORES):
        yc = y[cc * R_LOC : (cc + 1) * R_LOC]
        # yt[q, t, k, s] = yc[t*128 + s, k*128 + q]
        a = yc.reshape(R_TILES, 128, K_TILES, 128)
        yct = np.ascontiguousarray(a.transpose(3, 0, 2, 1).astype(MM_NP))
        y2 = np.einsum("rd,rd->r", yc, yc, dtype=np.float32) - Y2_SHIFT
        y2t = np.ascontiguousarray(y2.reshape(R_TILES, 128).T)
        in_maps.append({"xt": xt, "yt": yct, "y2t": y2t})
    return in_maps


def _postprocess(x: np.ndarray, accs: np.ndarray, hbds: np.ndarray) -> np.ndarray:
    """accs: [NCORES, 128, PA] partial mins; hbds: [NCORES, 128, T, PB]."""
    x2 = np.einsum("pd,pd->p", x, x, dtype=np.float32)
    ma = accs.astype(np.float32).min(axis=(0, 1)) + Y2_SHIFT    # [PA]
    mb = hbds.astype(np.float32).min(axis=(0, 1, 2)) + Y2_SHIFT  # [PB]
    m = np.concatenate([ma, mb])
    sq_min = np.float32((x2 + m).min())
    return np.sqrt(np.maximum(sq_min, np.float32(0.0)), dtype=np.float32)


def kernel(
    predicted_transaction_company: np.ndarray,
    future_transaction_companies_inc_current_data: np.ndarray,
) -> np.ndarray:
    x = np.asarray(predicted_transaction_company, dtype=np.float32)[0]
    y = np.asarray(future_transaction_companies_inc_current_data, dtype=np.float32)[0]

    nc = _get_module()
    in_maps = _prepare_inputs(x, y)
    res = bass_utils.run_bass_kernel_spmd(nc, in_maps, core_ids=list(range(NCORES)))
    accs = np.stack([r["out"] for r in res.results])
    hbds = np.stack([r["hbd"] for r in res.results])
    return _postprocess(x, accs, hbds)
